# revision 34
# baseline (speedup 1.0000x reference)
"""AttnDecoderRNN single-step on 8 Trainium2 NeuronCores (tensor parallel).

Math (faithful to the reference, including the softmax-over-singleton bug):
    embedded     = embeddings_index[input_id]                  (H,)
    attn_weights = ones(1, S)                                  (softmax of (S,1) over axis -1)
    attn_applied = column-sums of encoder_outputs              (1, H)
    x            = relu([embedded | attn_applied] @ comb_w.T + comb_b)
    h_new        = GRU_step(x, h0)                             (1, H)
    log_probs    = log_softmax(h_new @ out_w.T + out_b)        (1, V)

Sharding (8 cores), 3 collectives total:
    - attn column-sums + attn_combine are CONTRACTION-sharded: core c owns a
      256-wide slice of the embed dims and of the attn dims, computes local
      column sums (DVE, exact) and a partial u = cat @ comb_w.T; one
      AllReduce(add) of u. relu(u + comb_b) then runs replicated (tiny).
    - GRU output-dim sharded; the h0-side gate matmuls (gh) need no x and run
      before the first collective barrier completes. AllGather of h_new.
    - vocab projection rows-sharded (tensor parallel over V); per-core
      sum(exp(logits)) scalars AllGather -> logZ; each core writes its shard.

The first collective pays a global barrier absorbing multi-core launch skew
(~40-45us in this harness); all independent weight loads and the gh/colsum/
comb-partial compute are scheduled inside that window.

Precision: chain matmuls run as float32r (fp22 on TensorE, 1 cyc/row vs
fp32's 4); the dominant vocab projection streams bf16 weights (exact bf16
products, fp32 accumulate). Colsum is exact f32 on DVE.
"""

import numpy as np

H = 2048
S = 2048
V = 50257
NC = 8
P = 128
HC = H // NC          # 256  per-core slice
KC = H // P           # 16   contraction chunks of 128
VS = 6400             # per-core padded vocab rows (VS * NC = 51200 >= V)
PAD_B = -30.0         # bias for padded vocab rows: exp(-30) ~ 9e-14
VG = [(0, 2048), (2048, 2048), (4096, 2048), (6144, 256)]

_CACHE = {}


def _build():
    import concourse.mybir as mybir
    import concourse.tile as tile
    from concourse import bacc

    fp = mybir.dt.float32
    bf = mybir.dt.bfloat16
    AF = mybir.ActivationFunctionType
    ALU = mybir.AluOpType
    RG = [list(range(NC))]
    fr = mybir.dt.float32r

    nc = bacc.Bacc(
        "TRN2",
        target_bir_lowering=False,
        debug=False,
        enable_asserts=False,
        num_devices=NC,
    )

    ones_in = nc.dram_tensor("ones_in", [P], fr, kind="ExternalInput")
    # encoder columns I2_c, transposed: (HC, S) f32 (DVE column-sum source)
    enccT = nc.dram_tensor("enccT", [HC, S], fp, kind="ExternalInput")
    emb_sl = nc.dram_tensor("emb_sl", [HC], fr, kind="ExternalInput")
    h0 = nc.dram_tensor("h0", [H], fr, kind="ExternalInput")
    h0_sl = nc.dram_tensor("h0_sl", [HC], fp, kind="ExternalInput")
    # comb_w columns (I_c | 2048+I2_c), transposed: (2*HC, H)
    cwu = nc.dram_tensor("cwu", [2 * HC, H], fr, kind="ExternalInput")
    comb_b = nc.dram_tensor("comb_b", [H], fp, kind="ExternalInput")
    ghw = nc.dram_tensor("ghw", [H, 3 * HC], fr, kind="ExternalInput")
    gxw = nc.dram_tensor("gxw", [H, 3 * HC], fr, kind="ExternalInput")
    gru_b = nc.dram_tensor("gru_b", [6 * HC], fr, kind="ExternalInput")
    out_wT = nc.dram_tensor("out_wT", [H, VS], bf, kind="ExternalInput")
    out_b = nc.dram_tensor("out_b", [VS], fr, kind="ExternalInput")

    out_lp = nc.dram_tensor("out_lp", [VS], fp, kind="ExternalOutput")
    out_h = nc.dram_tensor("out_h", [H], fp, kind="ExternalOutput")

    def row(t):
        return t.ap().rearrange("(a n) -> a n", a=1)

    def kmaj(ap2d):
        # flat (C*P,) -> [P, C] with chunk c in column c
        return ap2d.rearrange("a (c k) -> k (a c)", k=P)

    with tile.TileContext(nc) as tc:
        with (
            tc.tile_pool(name="c1", bufs=1) as c1,
            tc.tile_pool(name="wvp", bufs=8) as wvp,
            tc.tile_pool(name="encp", bufs=2) as encp,
            tc.tile_pool(name="cwup", bufs=2) as cwup,
            tc.tile_pool(name="ghp", bufs=2) as ghp,
            tc.tile_pool(name="gxp", bufs=16) as gxp,
            tc.tile_pool(name="dram", bufs=1, space="DRAM") as dp,
        ):
            sy = nc.sync
            ve = nc.vector
            se = nc.scalar

            # ---- constants & small loads ----
            ones = c1.tile([P, 1], fr, tag="ones")
            sy.dma_start(ones[:], ones_in.ap().rearrange("(k a) -> k a", a=1))
            cat4 = c1.tile([P, 4], fr, tag="cat4")
            sy.dma_start(cat4[:, 0:2], kmaj(row(emb_sl)))
            h0_km = c1.tile([P, KC], fr, tag="h0_km")
            sy.dma_start(h0_km[:], kmaj(row(h0)))
            h0sl = c1.tile([1, HC], fp, tag="h0sl")
            sy.dma_start(h0sl[:], row(h0_sl))
            combb_km = c1.tile([P, KC], fp, tag="combb_km")
            sy.dma_start(combb_km[:], kmaj(row(comb_b)))
            grub = c1.tile([1, 6 * HC], fr, tag="grub")
            sy.dma_start(grub[:], row(gru_b))
            outb = c1.tile([1, VS], fr, tag="outb")
            sy.dma_start(outb[:], row(out_b))

            # Warm the natural_log_exp ACT table set so the tail's Ln does
            # not pay a ~2.7us table switch: Ln(0*x + 1) == 0, discarded.
            warm = c1.tile([1, 1], fp, tag="warm")
            se.activation(warm[:], h0sl[0:1, 0:1], AF.Ln, bias=1.0, scale=0.0)

            # DRAM bounce buffers
            cc_ac = dp.tile([1, 2 * P], fp, tag="cc_ac")     # attn k-major hop
            cc_u_i = dp.tile([1, H], fp, tag="cc_u_i")
            cc_u_o = dp.tile([1, H], fp, tag="cc_u_o")
            cc_x_s = dp.tile([1, H], fp, tag="cc_x_s")       # x k-major hop
            cc_h_i = dp.tile([1, HC], fp, tag="cc_h_i")
            cc_h_o = dp.tile([1, H], fp, tag="cc_h_o")
            cc_s_i = dp.tile([1, 8], fp, tag="cc_s_i")
            cc_s_o = dp.tile([1, 8 * NC], fp, tag="cc_s_o")

            with tc.tile_pool(name="psc", bufs=1, space="PSUM") as psc:
                # ---- local attn column-sums (DVE, exact f32) ----
                attn_f = c1.tile([P, 2], fp, tag="attn_f")
                for t in range(2):
                    et = encp.tile([P, S], fp, tag="enc")
                    sy.dma_start(et[:], enccT.ap()[t * P:(t + 1) * P, :])
                    ve.tensor_reduce(
                        attn_f[:, t:t + 1], et[:], axis=mybir.AxisListType.X,
                        op=ALU.add,
                    )
                # f32r hop through DRAM (PE needs float32r-typed operands)
                se.dma_start(kmaj(cc_ac[:]), attn_f[:])
                se.dma_start(cat4[:, 2:4], kmaj(cc_ac[:]).bitcast(fr))

                # ---- partial u = cat_c @ comb_w_c.T (contraction shard) ----
                u_ps = psc.tile([1, H], fp, tag="ups")
                for kc4 in range(4):
                    cwt = cwup.tile([P, H], fr, tag="cw")
                    sy.dma_start(cwt[:], cwu.ap()[kc4 * P:(kc4 + 1) * P, :])
                    for ns in range(4):
                        nc.tensor.matmul(
                            u_ps[0:1, ns * 512:(ns + 1) * 512],
                            lhsT=cat4[:, kc4:kc4 + 1],
                            rhs=cwt[:, ns * 512:(ns + 1) * 512],
                            start=(kc4 == 0),
                            stop=(kc4 == 3),
                        )
                u_sb = c1.tile([1, H], fp, tag="u_sb")
                se.copy(u_sb[:], u_ps[:])
                se.dma_start(cc_u_i[:], u_sb[:])
                nc.gpsimd.collective_compute(
                    "AllReduce", ALU.add, replica_groups=RG,
                    ins=[cc_u_i.opt()], outs=[cc_u_o.opt()],
                )

                # ---- GRU gh half (h0 side): runs inside the barrier window
                # PSUM start=True clears accumulate-flags for the WHOLE
                # 512-f32 bank, so every matmul region is bank-aligned:
                # bank0 [0:512]     = gx_r|gx_z
                # bank1 [512:768]   = gx_n   (cols 768:1024 unused)
                # bank2 [1024:1536] = gh_r|gh_z
                # bank3 [1536:1792] = gh_n   (cols 1792:2048 unused)
                g_ps = psc.tile([1, 2048], fp, tag="gps")
                for off, w, woff in ((0, 512, 0), (512, 256, 512),
                                     (1024, 512, 768), (1536, 256, 1280)):
                    nc.tensor.matmul(
                        g_ps[0:1, off:off + w], lhsT=ones[0:1, 0:1],
                        rhs=grub[0:1, woff:woff + w], start=True, stop=False,
                    )
                for kc in range(KC):
                    ght = ghp.tile([P, 3 * HC], fr, tag="ghw")
                    sy.dma_start(ght[:], ghw.ap()[kc * P:(kc + 1) * P, :])
                    for off, w, woff in ((1024, 512, 0), (1536, 256, 512)):
                        nc.tensor.matmul(
                            g_ps[0:1, off:off + w],
                            lhsT=h0_km[:, kc:kc + 1],
                            rhs=ght[:, woff:woff + w],
                            start=False,
                            stop=(kc == KC - 1),
                        )
                # gx weights fully resident before x arrives
                gxt = []
                for kc in range(KC):
                    gxt.append(gxp.tile([P, 3 * HC], fr, tag="gxw",
                                        name=f"gxt{kc}"))
                    sy.dma_start(gxt[kc][:], gxw.ap()[kc * P:(kc + 1) * P, :])

                # ---- post-AllReduce: x = relu(u + comb_b), k-major ----
                u_km = c1.tile([P, KC], fp, tag="u_km")
                se.dma_start(u_km[:], kmaj(cc_u_o[:]))
                xf_km = c1.tile([P, KC], fp, tag="xf_km")
                ve.tensor_add(xf_km[:], u_km[:], combb_km[:])
                ve.tensor_scalar_max(xf_km[:], xf_km[:], 0.0)
                se.dma_start(kmaj(cc_x_s[:]), xf_km[:])
                x_km = c1.tile([P, KC], fr, tag="x_km")
                se.dma_start(x_km[:], kmaj(cc_x_s[:]).bitcast(fr))

                # ---- GRU gx half + gates ----
                for kc in range(KC):
                    for off, w, woff in ((0, 512, 0), (512, 256, 512)):
                        nc.tensor.matmul(
                            g_ps[0:1, off:off + w],
                            lhsT=x_km[:, kc:kc + 1],
                            rhs=gxt[kc][:, woff:woff + w],
                            start=False,
                            stop=(kc == KC - 1),
                        )
                gh_sb = c1.tile([1, 3 * HC], fp, tag="gh_sb")
                se.copy(gh_sb[0:1, 0:512], g_ps[0:1, 1024:1536])
                se.copy(gh_sb[0:1, 512:768], g_ps[0:1, 1536:1792])
                rz_sb = c1.tile([1, 2 * HC], fp, tag="rz_sb")
                ve.tensor_add(rz_sb[:], g_ps[0:1, 0:2 * HC], gh_sb[0:1, 0:2 * HC])
                rzs = c1.tile([1, 2 * HC], fp, tag="rzs")
                se.activation(rzs[:], rz_sb[:], AF.Sigmoid)
                t1 = c1.tile([1, HC], fp, tag="t1")
                ve.tensor_mul(t1[:], rzs[0:1, 0:HC], gh_sb[0:1, 2 * HC:3 * HC])
                t2 = c1.tile([1, HC], fp, tag="t2")
                ve.tensor_add(t2[:], g_ps[0:1, 512:768], t1[:])
                n_sb = c1.tile([1, HC], fp, tag="n_sb")
                se.activation(n_sb[:], t2[:], AF.Tanh)
                t3 = c1.tile([1, HC], fp, tag="t3")
                ve.tensor_sub(t3[:], h0sl[:], n_sb[:])
                t4 = c1.tile([1, HC], fp, tag="t4")
                ve.tensor_mul(t4[:], rzs[0:1, HC:2 * HC], t3[:])
                hn_sb = c1.tile([1, HC], fp, tag="hn_sb")
                ve.tensor_add(hn_sb[:], n_sb[:], t4[:])
                se.dma_start(cc_h_i[:], hn_sb[:])
                nc.gpsimd.collective_compute(
                    "AllGather", ALU.bypass, replica_groups=RG,
                    ins=[cc_h_i.opt()], outs=[cc_h_o.opt()],
                )
                hN_km = c1.tile([P, KC], fp, tag="hN_km")
                se.dma_start(hN_km[:], kmaj(cc_h_o[:]))
                se.dma_start(row(out_h), cc_h_o[:])
                hN_bf = c1.tile([P, KC], bf, tag="hN_bf")
                ve.tensor_copy(hN_bf[:], hN_km[:])

            # ---- vocab projection (rows sharded) + fused sum(exp) ----
            logits_sb = c1.tile([1, VS], fp, tag="logits_sb")
            sg = c1.tile([1, 16], fp, tag="sg")
            gi = 0
            with tc.tile_pool(name="psv", bufs=4, space="PSUM") as psv:
                for g0, gw in VG:
                    if gw > 1024:
                        subs = [(g0, 1024), (g0 + 1024, gw - 1024)]
                    else:
                        subs = [(g0, gw)]
                    ps_list = [
                        psv.tile([1, sw], fp, tag="vps", name=f"vps_{s0}")
                        for (s0, sw) in subs
                    ]
                    for (s0, sw), pst in zip(subs, ps_list):
                        for ns0 in range(0, sw, 512):
                            w5 = min(512, sw - ns0)
                            nc.tensor.matmul(
                                pst[0:1, ns0:ns0 + w5],
                                lhsT=ones[0:1, 0:1],
                                rhs=outb[0:1, s0 + ns0:s0 + ns0 + w5],
                                start=True, stop=False,
                            )
                    for kc in range(KC):
                        wt = wvp.tile([P, gw], bf, tag="wv")
                        sy.dma_start(
                            wt[:], out_wT.ap()[kc * P:(kc + 1) * P, g0:g0 + gw]
                        )
                        for (s0, sw), pst in zip(subs, ps_list):
                            for ns0 in range(0, sw, 512):
                                w5 = min(512, sw - ns0)
                                nc.tensor.matmul(
                                    pst[0:1, ns0:ns0 + w5],
                                    lhsT=hN_bf[:, kc:kc + 1],
                                    rhs=wt[:, (s0 - g0) + ns0:(s0 - g0) + ns0 + w5],
                                    start=False,
                                    stop=(kc == KC - 1),
                                )
                    for (s0, sw), pst in zip(subs, ps_list):
                        ve.tensor_copy(logits_sb[0:1, s0:s0 + sw], pst[0:1, :])
                        for e0 in range(0, sw, 512):
                            ew = min(512, sw - e0)
                            ex = c1.tile([1, 512], fp, tag="ex", bufs=2,
                                         name=f"ex_{s0}_{e0}")
                            se.activation(
                                ex[0:1, 0:ew], pst[0:1, e0:e0 + ew], AF.Exp,
                                accum_out=sg[0:1, gi:gi + 1],
                            )
                            gi += 1

            # ---- logZ (AllGather of per-core sum-exp) + subtract + store
            sloc = c1.tile([1, 8], fp, tag="sloc")
            ve.memset(sloc[:], 0.0)
            ve.tensor_reduce(
                sloc[0:1, 0:1], sg[0:1, 0:gi], axis=mybir.AxisListType.X,
                op=ALU.add,
            )
            se.dma_start(cc_s_i[:], sloc[:])
            nc.gpsimd.collective_compute(
                "AllGather", ALU.bypass, replica_groups=RG,
                ins=[cc_s_i.opt()], outs=[cc_s_o.opt()],
            )
            s_sb = c1.tile([1, 8 * NC], fp, tag="s_sb")
            se.dma_start(s_sb[:], cc_s_o[:])
            s_tot = c1.tile([1, 1], fp, tag="s_tot")
            ve.tensor_reduce(
                s_tot[0:1, 0:1],
                s_sb[:].rearrange("a (r e) -> a r e", e=8)[:, :, 0],
                axis=mybir.AxisListType.X, op=ALU.add,
            )
            logz = c1.tile([1, 1], fp, tag="logz")
            se.activation(logz[:], s_tot[:], AF.Ln)
            ve.tensor_scalar_sub(logits_sb[:], logits_sb[:], logz[0:1, 0:1])
            se.dma_start(row(out_lp), logits_sb[:])

    nc.compile()
    return nc


def _get_compiled():
    if "nc" not in _CACHE:
        _CACHE["nc"] = _build()
    return _CACHE["nc"]


def _prep(inputs):
    import ml_dtypes

    f = np.float32
    input_id = int(np.asarray(inputs["input_id"]))
    hidden = np.ascontiguousarray(np.asarray(inputs["hidden"], f).reshape(H))
    enc = np.ascontiguousarray(np.asarray(inputs["encoder_outputs"], f))
    embeddings = np.asarray(inputs["embeddings_index"], f)
    comb_w = np.asarray(inputs["comb_w"], f)
    comb_bv = np.asarray(inputs["comb_b"], f)
    w_ih = np.asarray(inputs["w_ih"], f)
    w_hh = np.asarray(inputs["w_hh"], f)
    b_ih = np.asarray(inputs["b_ih"], f)
    b_hh = np.asarray(inputs["b_hh"], f)
    out_w = np.asarray(inputs["out_w"], f)
    out_bv = np.asarray(inputs["out_b"], f)

    emb_row = np.ascontiguousarray(embeddings[input_id])
    maps = []
    for c in range(NC):
        lo, hi = c * HC, (c + 1) * HC
        xsel = np.concatenate(
            [w_ih[lo:hi], w_ih[H + lo:H + hi], w_ih[2 * H + lo:2 * H + hi]],
            axis=0,
        )
        hsel = np.concatenate(
            [w_hh[lo:hi], w_hh[H + lo:H + hi], w_hh[2 * H + lo:2 * H + hi]],
            axis=0,
        )
        gb = np.concatenate(
            [b_ih[lo:hi], b_ih[H + lo:H + hi], b_ih[2 * H + lo:2 * H + hi],
             b_hh[lo:hi], b_hh[H + lo:H + hi], b_hh[2 * H + lo:2 * H + hi]],
        )
        v0 = c * VS
        nrows = min(VS, max(0, V - v0))
        wsh = np.zeros((VS, H), f)
        wsh[:nrows] = out_w[v0:v0 + nrows]
        wsh = wsh.astype(ml_dtypes.bfloat16)
        bsh = np.full((VS,), PAD_B, f)
        bsh[:nrows] = out_bv[v0:v0 + nrows]
        maps.append({
            "ones_in": np.ones((P,), f),
            "enccT": np.ascontiguousarray(enc[:, lo:hi].T),
            "emb_sl": np.ascontiguousarray(emb_row[lo:hi]),
            "h0": hidden,
            "h0_sl": np.ascontiguousarray(hidden[lo:hi]),
            "cwu": np.ascontiguousarray(
                np.concatenate([comb_w[:, lo:hi], comb_w[:, H + lo:H + hi]],
                               axis=1).T
            ),
            "comb_b": comb_bv,
            "ghw": np.ascontiguousarray(hsel.T),
            "gxw": np.ascontiguousarray(xsel.T),
            "gru_b": np.ascontiguousarray(gb),
            "out_wT": np.ascontiguousarray(wsh.T),
            "out_b": bsh,
        })
    return maps


def _assemble(results):
    lp = np.concatenate([results[c]["out_lp"] for c in range(NC)])[:V]
    log_probs = np.ascontiguousarray(lp.reshape(1, V))
    h_new = np.ascontiguousarray(results[0]["out_h"].reshape(1, 1, H))
    attn_weights = np.ones((1, S), np.float32)
    return log_probs, h_new, attn_weights


def _run(inputs, trace=False, trace_cores=None):
    import concourse.bass_utils as bass_utils

    nc = _get_compiled()
    maps = _prep(inputs)
    res = bass_utils.run_bass_kernel_spmd(
        nc, maps, core_ids=list(range(NC)), trace=trace, trace_cores=trace_cores,
    )
    return res


def kernel(**inputs):
    res = _run(inputs, trace=False)
    return _assemble(res.results)


# revision 35
# speedup vs baseline: 1.2128x; 1.2128x over previous
"""AttnDecoderRNN single-step on 8 Trainium2 NeuronCores (tensor parallel).

Math (faithful to the reference, including the softmax-over-singleton bug):
    embedded     = embeddings_index[input_id]                  (H,)
    attn_weights = ones(1, S)                                  (softmax of (S,1) over axis -1)
    attn_applied = column-sums of encoder_outputs              (1, H)
    x            = relu([embedded | attn_applied] @ comb_w.T + comb_b)
    h_new        = GRU_step(x, h0)                             (1, H)
    log_probs    = log_softmax(h_new @ out_w.T + out_b)        (1, V)

Sharding (8 cores), 3 collectives total:
    - attn column-sums + attn_combine are CONTRACTION-sharded: core c owns a
      256-wide slice of the embed dims and of the attn dims, computes local
      column sums (DVE, exact) and a partial u = cat @ comb_w.T; one
      AllReduce(add) of u. relu(u + comb_b) then runs replicated (tiny).
    - GRU output-dim sharded; the h0-side gate matmuls (gh) need no x and run
      before the first collective barrier completes. AllGather of h_new.
    - vocab projection rows-sharded (tensor parallel over V); per-core
      sum(exp(logits)) scalars AllGather -> logZ; each core writes its shard.

The first collective pays a global barrier absorbing multi-core launch skew
(~40-45us in this harness); all independent weight loads and the gh/colsum/
comb-partial compute are scheduled inside that window.

Precision: chain matmuls run as float32r (fp22 on TensorE, 1 cyc/row vs
fp32's 4); the dominant vocab projection streams bf16 weights (exact bf16
products, fp32 accumulate). Colsum is exact f32 on DVE.
"""

import numpy as np

H = 2048
S = 2048
V = 50257
NC = 8
P = 128
HC = H // NC          # 256  per-core slice
KC = H // P           # 16   contraction chunks of 128
VS = 6400             # per-core padded vocab rows (VS * NC = 51200 >= V)
PAD_B = -30.0         # bias for padded vocab rows: exp(-30) ~ 9e-14
VG = [(0, 2048), (2048, 2048), (4096, 2048), (6144, 256)]

_CACHE = {}


def _build():
    import concourse.mybir as mybir
    import concourse.tile as tile
    from concourse import bacc

    fp = mybir.dt.float32
    bf = mybir.dt.bfloat16
    AF = mybir.ActivationFunctionType
    ALU = mybir.AluOpType
    RG = [list(range(NC))]
    fr = mybir.dt.float32r

    nc = bacc.Bacc(
        "TRN2",
        target_bir_lowering=False,
        debug=False,
        enable_asserts=False,
        num_devices=NC,
    )

    ones_in = nc.dram_tensor("ones_in", [P], fr, kind="ExternalInput")
    # encoder columns I2_c, transposed: (HC, S) f32 (DVE column-sum source)
    enccT = nc.dram_tensor("enccT", [HC, S], fp, kind="ExternalInput")
    emb_sl = nc.dram_tensor("emb_sl", [HC], fr, kind="ExternalInput")
    h0 = nc.dram_tensor("h0", [H], fr, kind="ExternalInput")
    h0_sl = nc.dram_tensor("h0_sl", [HC], fp, kind="ExternalInput")
    # comb_w columns (I_c | 2048+I2_c), transposed: (2*HC, H)
    cwu = nc.dram_tensor("cwu", [2 * HC, H], fr, kind="ExternalInput")
    comb_b = nc.dram_tensor("comb_b", [H], fp, kind="ExternalInput")
    ghw = nc.dram_tensor("ghw", [H, 3 * HC], fr, kind="ExternalInput")
    gxw = nc.dram_tensor("gxw", [H, 3 * HC], fr, kind="ExternalInput")
    gru_b = nc.dram_tensor("gru_b", [6 * HC], fr, kind="ExternalInput")
    out_wT = nc.dram_tensor("out_wT", [H, VS], bf, kind="ExternalInput")
    out_b = nc.dram_tensor("out_b", [VS], fr, kind="ExternalInput")

    out_lp = nc.dram_tensor("out_lp", [VS], fp, kind="ExternalOutput")
    out_h = nc.dram_tensor("out_h", [H], fp, kind="ExternalOutput")

    def row(t):
        return t.ap().rearrange("(a n) -> a n", a=1)

    def kmaj(ap2d):
        # flat (P*C,) -> [P, C]: tile[k, c] = v[k*C + c]. Contraction dim d
        # maps to (k, c) = (d // C, d % C); weight rows are permuted on the
        # host to match, giving 64B-contiguous DMA runs per partition.
        return ap2d.rearrange("a (k c) -> k (a c)", k=P)

    with tile.TileContext(nc) as tc:
        with (
            tc.tile_pool(name="c1", bufs=1) as c1,
            tc.tile_pool(name="wvp", bufs=8) as wvp,
            tc.tile_pool(name="encp", bufs=2) as encp,
            tc.tile_pool(name="cwup", bufs=2) as cwup,
            tc.tile_pool(name="ghp", bufs=2) as ghp,
            tc.tile_pool(name="gxp", bufs=16) as gxp,
            tc.tile_pool(name="dram", bufs=1, space="DRAM") as dp,
        ):
            sy = nc.sync
            ve = nc.vector
            se = nc.scalar

            # ---- constants & small loads ----
            ones = c1.tile([P, 1], fr, tag="ones")
            sy.dma_start(ones[:], ones_in.ap().rearrange("(k a) -> k a", a=1))
            cat4 = c1.tile([P, 4], fr, tag="cat4")
            sy.dma_start(cat4[:, 0:2], kmaj(row(emb_sl)))
            h0_km = c1.tile([P, KC], fr, tag="h0_km")
            sy.dma_start(h0_km[:], kmaj(row(h0)))
            h0sl = c1.tile([1, HC], fp, tag="h0sl")
            sy.dma_start(h0sl[:], row(h0_sl))
            combb_km = c1.tile([P, KC], fp, tag="combb_km")
            sy.dma_start(combb_km[:], kmaj(row(comb_b)))
            grub = c1.tile([1, 6 * HC], fr, tag="grub")
            sy.dma_start(grub[:], row(gru_b))
            outb = c1.tile([1, VS], fr, tag="outb")
            sy.dma_start(outb[:], row(out_b))

            # Warm the natural_log_exp ACT table set so the tail's Ln does
            # not pay a ~2.7us table switch: Ln(0*x + 1) == 0, discarded.
            warm = c1.tile([1, 1], fp, tag="warm")
            se.activation(warm[:], h0sl[0:1, 0:1], AF.Ln, bias=1.0, scale=0.0)

            # DRAM bounce buffers
            cc_ac = dp.tile([1, 2 * P], fp, tag="cc_ac")     # attn k-major hop
            cc_u_i = dp.tile([1, H], fp, tag="cc_u_i")
            cc_u_o = dp.tile([1, H], fp, tag="cc_u_o")
            cc_x_s = dp.tile([1, H], fp, tag="cc_x_s")       # x k-major hop
            cc_h_i = dp.tile([1, HC], fp, tag="cc_h_i")
            cc_h_o = dp.tile([1, H], fp, tag="cc_h_o")
            cc_s_i = dp.tile([1, 8], fp, tag="cc_s_i")
            cc_s_o = dp.tile([1, 8 * NC], fp, tag="cc_s_o")

            with tc.tile_pool(name="psc", bufs=1, space="PSUM") as psc:
                # ---- local attn column-sums (DVE, exact f32) ----
                attn_f = c1.tile([P, 2], fp, tag="attn_f")
                for t in range(2):
                    et = encp.tile([P, S], fp, tag="enc")
                    sy.dma_start(et[:], enccT.ap()[t * P:(t + 1) * P, :])
                    ve.tensor_reduce(
                        attn_f[:, t:t + 1], et[:], axis=mybir.AxisListType.X,
                        op=ALU.add,
                    )
                # f32r hop through DRAM (PE needs float32r-typed operands)
                se.dma_start(kmaj(cc_ac[:]), attn_f[:])
                se.dma_start(cat4[:, 2:4], kmaj(cc_ac[:]).bitcast(fr))

                # ---- partial u = cat_c @ comb_w_c.T (contraction shard) ----
                u_ps = psc.tile([1, H], fp, tag="ups")
                for kc4 in range(4):
                    cwt = cwup.tile([P, H], fr, tag="cw")
                    sy.dma_start(cwt[:], cwu.ap()[kc4 * P:(kc4 + 1) * P, :])
                    for ns in range(4):
                        nc.tensor.matmul(
                            u_ps[0:1, ns * 512:(ns + 1) * 512],
                            lhsT=cat4[:, kc4:kc4 + 1],
                            rhs=cwt[:, ns * 512:(ns + 1) * 512],
                            start=(kc4 == 0),
                            stop=(kc4 == 3),
                        )
                u_sb = c1.tile([1, H], fp, tag="u_sb")
                se.copy(u_sb[:], u_ps[:])
                se.dma_start(cc_u_i[:], u_sb[:])
                nc.gpsimd.collective_compute(
                    "AllReduce", ALU.add, replica_groups=RG,
                    ins=[cc_u_i.opt()], outs=[cc_u_o.opt()],
                )

                # ---- GRU gh half (h0 side): runs inside the barrier window
                # PSUM start=True clears accumulate-flags for the WHOLE
                # 512-f32 bank, so every matmul region is bank-aligned:
                # bank0 [0:512]     = gx_r|gx_z
                # bank1 [512:768]   = gx_n   (cols 768:1024 unused)
                # bank2 [1024:1536] = gh_r|gh_z
                # bank3 [1536:1792] = gh_n   (cols 1792:2048 unused)
                g_ps = psc.tile([1, 2048], fp, tag="gps")
                for off, w, woff in ((0, 512, 0), (512, 256, 512),
                                     (1024, 512, 768), (1536, 256, 1280)):
                    nc.tensor.matmul(
                        g_ps[0:1, off:off + w], lhsT=ones[0:1, 0:1],
                        rhs=grub[0:1, woff:woff + w], start=True, stop=False,
                    )
                # gx weights fully resident before x arrives (issued
                # ahead of the ghw stream so ghw slot-waits cannot block them
                # in the sync DGE ring)
                gxt = []
                for kc in range(KC):
                    gxt.append(gxp.tile([P, 3 * HC], fr, tag="gxw",
                                        name=f"gxt{kc}"))
                    sy.dma_start(gxt[kc][:], gxw.ap()[kc * P:(kc + 1) * P, :])
                for kc in range(KC):
                    ght = ghp.tile([P, 3 * HC], fr, tag="ghw")
                    sy.dma_start(ght[:], ghw.ap()[kc * P:(kc + 1) * P, :])
                    for off, w, woff in ((1024, 512, 0), (1536, 256, 512)):
                        nc.tensor.matmul(
                            g_ps[0:1, off:off + w],
                            lhsT=h0_km[:, kc:kc + 1],
                            rhs=ght[:, woff:woff + w],
                            start=False,
                            stop=(kc == KC - 1),
                        )

                # ---- post-AllReduce: x = relu(u + comb_b), k-major ----
                u_km = c1.tile([P, KC], fp, tag="u_km")
                se.dma_start(u_km[:], kmaj(cc_u_o[:]))
                xf_km = c1.tile([P, KC], fp, tag="xf_km")
                ve.tensor_add(xf_km[:], u_km[:], combb_km[:])
                ve.tensor_scalar_max(xf_km[:], xf_km[:], 0.0)
                se.dma_start(kmaj(cc_x_s[:]), xf_km[:])
                x_km = c1.tile([P, KC], fr, tag="x_km")
                se.dma_start(x_km[:], kmaj(cc_x_s[:]).bitcast(fr))

                # ---- GRU gx half + gates ----
                for kc in range(KC):
                    for off, w, woff in ((0, 512, 0), (512, 256, 512)):
                        nc.tensor.matmul(
                            g_ps[0:1, off:off + w],
                            lhsT=x_km[:, kc:kc + 1],
                            rhs=gxt[kc][:, woff:woff + w],
                            start=False,
                            stop=(kc == KC - 1),
                        )
                gh_sb = c1.tile([1, 3 * HC], fp, tag="gh_sb")
                se.copy(gh_sb[0:1, 0:512], g_ps[0:1, 1024:1536])
                se.copy(gh_sb[0:1, 512:768], g_ps[0:1, 1536:1792])
                rz_sb = c1.tile([1, 2 * HC], fp, tag="rz_sb")
                ve.tensor_add(rz_sb[:], g_ps[0:1, 0:2 * HC], gh_sb[0:1, 0:2 * HC])
                rzs = c1.tile([1, 2 * HC], fp, tag="rzs")
                se.activation(rzs[:], rz_sb[:], AF.Sigmoid)
                t1 = c1.tile([1, HC], fp, tag="t1")
                ve.tensor_mul(t1[:], rzs[0:1, 0:HC], gh_sb[0:1, 2 * HC:3 * HC])
                t2 = c1.tile([1, HC], fp, tag="t2")
                ve.tensor_add(t2[:], g_ps[0:1, 512:768], t1[:])
                n_sb = c1.tile([1, HC], fp, tag="n_sb")
                se.activation(n_sb[:], t2[:], AF.Tanh)
                t3 = c1.tile([1, HC], fp, tag="t3")
                ve.tensor_sub(t3[:], h0sl[:], n_sb[:])
                t4 = c1.tile([1, HC], fp, tag="t4")
                ve.tensor_mul(t4[:], rzs[0:1, HC:2 * HC], t3[:])
                hn_sb = c1.tile([1, HC], fp, tag="hn_sb")
                ve.tensor_add(hn_sb[:], n_sb[:], t4[:])
                se.dma_start(cc_h_i[:], hn_sb[:])
                nc.gpsimd.collective_compute(
                    "AllGather", ALU.bypass, replica_groups=RG,
                    ins=[cc_h_i.opt()], outs=[cc_h_o.opt()],
                )
                hN_km = c1.tile([P, KC], fp, tag="hN_km")
                se.dma_start(hN_km[:], kmaj(cc_h_o[:]))
                se.dma_start(row(out_h), cc_h_o[:])
                hN_bf = c1.tile([P, KC], bf, tag="hN_bf")
                ve.tensor_copy(hN_bf[:], hN_km[:])

            # ---- vocab projection (rows sharded) + fused sum(exp) ----
            logits_sb = c1.tile([1, VS], fp, tag="logits_sb")
            sg = c1.tile([1, 16], fp, tag="sg")
            gi = 0
            with tc.tile_pool(name="psv", bufs=4, space="PSUM") as psv:
                for g0, gw in VG:
                    if gw > 1024:
                        subs = [(g0, 1024), (g0 + 1024, gw - 1024)]
                    else:
                        subs = [(g0, gw)]
                    ps_list = [
                        psv.tile([1, sw], fp, tag="vps", name=f"vps_{s0}")
                        for (s0, sw) in subs
                    ]
                    for (s0, sw), pst in zip(subs, ps_list):
                        for ns0 in range(0, sw, 512):
                            w5 = min(512, sw - ns0)
                            nc.tensor.matmul(
                                pst[0:1, ns0:ns0 + w5],
                                lhsT=ones[0:1, 0:1],
                                rhs=outb[0:1, s0 + ns0:s0 + ns0 + w5],
                                start=True, stop=False,
                            )
                    for kc in range(KC):
                        wt = wvp.tile([P, gw], bf, tag="wv")
                        sy.dma_start(
                            wt[:], out_wT.ap()[kc * P:(kc + 1) * P, g0:g0 + gw]
                        )
                        for (s0, sw), pst in zip(subs, ps_list):
                            for ns0 in range(0, sw, 512):
                                w5 = min(512, sw - ns0)
                                nc.tensor.matmul(
                                    pst[0:1, ns0:ns0 + w5],
                                    lhsT=hN_bf[:, kc:kc + 1],
                                    rhs=wt[:, (s0 - g0) + ns0:(s0 - g0) + ns0 + w5],
                                    start=False,
                                    stop=(kc == KC - 1),
                                )
                    for (s0, sw), pst in zip(subs, ps_list):
                        ve.tensor_copy(logits_sb[0:1, s0:s0 + sw], pst[0:1, :])
                        for e0 in range(0, sw, 512):
                            ew = min(512, sw - e0)
                            ex = c1.tile([1, 512], fp, tag="ex", bufs=2,
                                         name=f"ex_{s0}_{e0}")
                            se.activation(
                                ex[0:1, 0:ew], pst[0:1, e0:e0 + ew], AF.Exp,
                                accum_out=sg[0:1, gi:gi + 1],
                            )
                            gi += 1

            # ---- logZ (AllGather of per-core sum-exp) + subtract + store
            sloc = c1.tile([1, 8], fp, tag="sloc")
            ve.memset(sloc[:], 0.0)
            ve.tensor_reduce(
                sloc[0:1, 0:1], sg[0:1, 0:gi], axis=mybir.AxisListType.X,
                op=ALU.add,
            )
            se.dma_start(cc_s_i[:], sloc[:])
            nc.gpsimd.collective_compute(
                "AllGather", ALU.bypass, replica_groups=RG,
                ins=[cc_s_i.opt()], outs=[cc_s_o.opt()],
            )
            s_sb = c1.tile([1, 8 * NC], fp, tag="s_sb")
            se.dma_start(s_sb[:], cc_s_o[:])
            s_tot = c1.tile([1, 1], fp, tag="s_tot")
            ve.tensor_reduce(
                s_tot[0:1, 0:1],
                s_sb[:].rearrange("a (r e) -> a r e", e=8)[:, :, 0],
                axis=mybir.AxisListType.X, op=ALU.add,
            )
            logz = c1.tile([1, 1], fp, tag="logz")
            se.activation(logz[:], s_tot[:], AF.Ln)
            ve.tensor_scalar_sub(logits_sb[:], logits_sb[:], logz[0:1, 0:1])
            se.dma_start(row(out_lp), logits_sb[:])

    nc.compile()
    return nc


def _get_compiled():
    if "nc" not in _CACHE:
        _CACHE["nc"] = _build()
    return _CACHE["nc"]


def _perm(n):
    # perm[c*128 + k] = k*(n//128) + c : slab row order for kmaj mapping
    return np.ascontiguousarray(np.arange(n).reshape(128, n // 128).T).reshape(-1)


_P2048 = None
_P256 = None


def _prep(inputs):
    import ml_dtypes

    global _P2048, _P256
    if _P2048 is None:
        _P2048 = _perm(2048)
        _P256 = _perm(256)

    f = np.float32
    input_id = int(np.asarray(inputs["input_id"]))
    hidden = np.ascontiguousarray(np.asarray(inputs["hidden"], f).reshape(H))
    enc = np.ascontiguousarray(np.asarray(inputs["encoder_outputs"], f))
    embeddings = np.asarray(inputs["embeddings_index"], f)
    comb_w = np.asarray(inputs["comb_w"], f)
    comb_bv = np.asarray(inputs["comb_b"], f)
    w_ih = np.asarray(inputs["w_ih"], f)
    w_hh = np.asarray(inputs["w_hh"], f)
    b_ih = np.asarray(inputs["b_ih"], f)
    b_hh = np.asarray(inputs["b_hh"], f)
    out_w = np.asarray(inputs["out_w"], f)
    out_bv = np.asarray(inputs["out_b"], f)

    emb_row = np.ascontiguousarray(embeddings[input_id])
    maps = []
    for c in range(NC):
        lo, hi = c * HC, (c + 1) * HC
        xsel = np.concatenate(
            [w_ih[lo:hi], w_ih[H + lo:H + hi], w_ih[2 * H + lo:2 * H + hi]],
            axis=0,
        )
        hsel = np.concatenate(
            [w_hh[lo:hi], w_hh[H + lo:H + hi], w_hh[2 * H + lo:2 * H + hi]],
            axis=0,
        )
        gb = np.concatenate(
            [b_ih[lo:hi], b_ih[H + lo:H + hi], b_ih[2 * H + lo:2 * H + hi],
             b_hh[lo:hi], b_hh[H + lo:H + hi], b_hh[2 * H + lo:2 * H + hi]],
        )
        v0 = c * VS
        nrows = min(VS, max(0, V - v0))
        wsh = np.zeros((VS, H), f)
        wsh[:nrows] = out_w[v0:v0 + nrows]
        wsh = wsh.astype(ml_dtypes.bfloat16)
        bsh = np.full((VS,), PAD_B, f)
        bsh[:nrows] = out_bv[v0:v0 + nrows]
        maps.append({
            "ones_in": np.ones((P,), f),
            "enccT": np.ascontiguousarray(enc[:, lo:hi].T[_P256]),
            "emb_sl": np.ascontiguousarray(emb_row[lo:hi]),
            "h0": hidden,
            "h0_sl": np.ascontiguousarray(hidden[lo:hi]),
            "cwu": np.ascontiguousarray(np.concatenate(
                [comb_w[:, lo:hi].T[_P256], comb_w[:, H + lo:H + hi].T[_P256]],
                axis=0)
            ),
            "comb_b": comb_bv,
            "ghw": np.ascontiguousarray(hsel.T[_P2048]),
            "gxw": np.ascontiguousarray(xsel.T[_P2048]),
            "gru_b": np.ascontiguousarray(gb),
            "out_wT": np.ascontiguousarray(wsh.T[_P2048]),
            "out_b": bsh,
        })
    return maps


def _assemble(results):
    lp = np.concatenate([results[c]["out_lp"] for c in range(NC)])[:V]
    log_probs = np.ascontiguousarray(lp.reshape(1, V))
    h_new = np.ascontiguousarray(results[0]["out_h"].reshape(1, 1, H))
    attn_weights = np.ones((1, S), np.float32)
    return log_probs, h_new, attn_weights


def _run(inputs, trace=False, trace_cores=None):
    import concourse.bass_utils as bass_utils

    nc = _get_compiled()
    maps = _prep(inputs)
    res = bass_utils.run_bass_kernel_spmd(
        nc, maps, core_ids=list(range(NC)), trace=trace, trace_cores=trace_cores,
    )
    return res


def kernel(**inputs):
    res = _run(inputs, trace=False)
    return _assemble(res.results)
